# revision 11
# baseline (speedup 1.0000x reference)
"""Trainium2 Bass kernel for nn_ChannelGroupedMemory (scatter_memory).

Computation per group g (G=4096 groups, D=512, M=32 slots, K=3):
    att[g, m]   = sum_d q[g, d] * W[g, d, m]
    p[g]        = softmax(att[g] / temp[g])          (temp from temp_logit, shift)
    vals, idx   = top_k(p[g], 3)                     (descending)
    retrieved[g]= sum_k vals[k] * W[g, :, idx[k]]

Strategy (8 NeuronCores, groups sharded 512/core, memory-bound):
  - Host: transpose W -> W_T [G, M, D] so each memory slot is a contiguous
    2 KiB row; precompute invtemp[g] = 1/(temp[g]+1e-8) (f32, monotone -> does
    not perturb top-k order), and gbase[g] = 32*g_local row offsets.
  - Device, per block of 128 groups (group on partition):
      * DMA W_T block [128, 16K] (64 KiB contiguous per partition), 4 quarters
      * att via 32x DVE tensor_tensor_reduce (fused mult+reduce, f32)
      * softmax: DVE max (negated), ACT Exp(scale=invtemp, bias=-max*invtemp,
        accum_out=rowsum), DVE reciprocal + tensor_scalar
      * top-3: DVE Max8 + MaxIndex (descending, lowest-index tie-break)
      * gather the 3 chosen rows per group straight from HBM via
        gpsimd.dma_gather (row index = gbase + idx), then 3 per-partition
        scalar FMAs -> retrieved
All f32 on-device: top-3 ordering is only sensitive to att error (~3e-6)
vs. min rank3/4 gap 1.8e-5 across all groups of this input distribution.
"""

import os
import sys

import numpy as np

for _p in ("/opt/trn_rl_repo",):
    if _p not in sys.path:
        sys.path.insert(0, _p)

N_CORES = 8
G_TOT, D, M, K = 4096, 512, 32, 3
G = G_TOT // N_CORES          # 512 groups per core
BLK = 128                     # groups per block (partition dim)
NBLK = G // BLK               # 4 blocks per core
MQ = 4                        # W quarters per block (8 m-slots each)

_CACHE = {}
LAST_RESULTS = None           # BassKernelResults of the most recent HW run


def build_program():
    """Build (once) the per-core Bass/Tile program. SPMD: all 8 cores run
    this same program on different group shards."""
    if "nc" in _CACHE:
        return _CACHE["nc"]

    from contextlib import ExitStack

    import concourse.bass as bass
    import concourse.tile as tile
    from concourse import bacc, library_config, mybir

    f32 = mybir.dt.float32
    i16 = mybir.dt.int16
    i32 = mybir.dt.int32
    u32 = mybir.dt.uint32
    ALU = mybir.AluOpType
    ACTF = mybir.ActivationFunctionType

    nc = bacc.Bacc(
        "TRN2",
        target_bir_lowering=False,
        debug=False,
        enable_asserts=False,
        num_devices=N_CORES,
    )

    q_d = nc.dram_tensor("q", [G, D], f32, kind="ExternalInput").ap()
    wt_d = nc.dram_tensor("wt", [G, M, D], f32, kind="ExternalInput").ap()
    invt_d = nc.dram_tensor("invtemp", [G], f32, kind="ExternalInput").ap()
    gb_d = nc.dram_tensor("gbase", [G], f32, kind="ExternalInput").ap()
    ret_d = nc.dram_tensor("retrieved", [G, D], f32, kind="ExternalOutput").ap()
    idx_d = nc.dram_tensor("idx", [G, K], i32, kind="ExternalOutput").ap()
    vals_d = nc.dram_tensor("vals", [G, K], f32, kind="ExternalOutput").ap()

    wt_flat = wt_d.rearrange("g m d -> (g m) d")
    # per-block DRAM bounce scratch for the gather-index wrap layout
    scr_d = [
        nc.dram_tensor(f"scr{b}", [K * BLK], i16).ap() for b in range(NBLK)
    ]

    with tile.TileContext(nc) as tc:
        with ExitStack() as ctx:
            nc.gpsimd.load_library(library_config.mlp)

            wq_pool = ctx.enter_context(tc.tile_pool(name="wq", bufs=2))
            small = ctx.enter_context(tc.tile_pool(name="small", bufs=2))
            const_p = ctx.enter_context(tc.tile_pool(name="const", bufs=1))
            prod_p = ctx.enter_context(tc.tile_pool(name="prod", bufs=2))
            gath_p = ctx.enter_context(tc.tile_pool(name="gath", bufs=2))
            acc_p = ctx.enter_context(tc.tile_pool(name="acc", bufs=2))

            # per-partition constants: [p, b] layouts of invtemp / gbase
            invt_t = const_p.tile([BLK, NBLK], f32, name="invt_t")
            nc.scalar.dma_start(
                invt_t[:], invt_d.rearrange("(b p) -> p b", p=BLK)
            )
            gb_t = const_p.tile([BLK, NBLK], f32, name="gb_t")
            nc.scalar.dma_start(gb_t[:], gb_d.rearrange("(b p) -> p b", p=BLK))

            # software pipeline: block b's gather-dependent FMA + retrieved
            # store are emitted during block b+1's section, so the in-order
            # DVE stream never stalls on the gather DMA chain.
            pending = {}

            def finish_block(bb):
                g0p = bb * BLK
                gath_p_, mx8_p = pending.pop(bb)
                acc = acc_p.tile([BLK, D], f32, name=f"acc_b{bb}", tag="acc")
                nc.vector.tensor_scalar(
                    out=acc[:],
                    in0=gath_p_[:, 0, :],
                    scalar1=mx8_p[:, 0:1],
                    scalar2=None,
                    op0=ALU.mult,
                )
                for k in (1, 2):
                    nc.vector.scalar_tensor_tensor(
                        out=acc[:],
                        in0=gath_p_[:, k, :],
                        scalar=mx8_p[:, k : k + 1],
                        in1=acc[:],
                        op0=ALU.mult,
                        op1=ALU.add,
                    )
                nc.scalar.dma_start(ret_d[g0p : g0p + BLK, :], acc[:])

            for b in range(NBLK):
                g0 = b * BLK
                # q first: every att op needs it, and the Sync HWDGE ring is
                # FIFO — q behind the W stream would stall each block's start
                q_t = small.tile([BLK, D], f32, name=f"q_b{b}", tag="q_t")
                nc.sync.dma_start(q_t[:], q_d[g0 : g0 + BLK, :])
                # ---- W block load (2 halves of 16 m-slots: 4.2 MB DMAs) ----
                wq = []
                for qi in range(2):
                    w_t = wq_pool.tile(
                        [BLK, 16 * D], f32, name=f"w{qi}_b{b}", tag=f"wq{qi}"
                    )
                    nc.sync.dma_start(
                        w_t[:], wt_d[g0 : g0 + BLK, qi * 16 : (qi + 1) * 16, :]
                    )
                    wq.append(w_t)

                # ---- att[g, m] = <q[g], W_T[g, m, :]> ----
                att = small.tile([BLK, M], f32, name=f"att_b{b}", tag="att")
                for m in range(M):
                    # fused multiply+row-reduce on DVE:
                    # prod = (W_m + 0) * q ; att[:, m] = sum(prod)
                    # (tensor_tensor_reduce is not supported by the
                    # neuronxcc path; InstTensorScalarPtr is)
                    prod = prod_p.tile([BLK, D], f32, name=f"pr_b{b}_{m}", tag="prod")
                    nc.vector.scalar_tensor_tensor(
                        out=prod[:],
                        in0=wq[m // 16][:, (m % 16) * D : (m % 16 + 1) * D],
                        scalar=0.0,
                        in1=q_t[:],
                        op0=ALU.add,
                        op1=ALU.mult,
                        accum_out=att[:, m : m + 1],
                    )

                # ---- softmax(att * invtemp) ----
                negmx = small.tile([BLK, 1], f32, name=f"negmx_b{b}", tag="negmx")
                nc.vector.tensor_reduce(
                    out=negmx[:],
                    in_=att[:],
                    axis=mybir.AxisListType.X,
                    op=ALU.max,
                    negate=True,
                )
                bias = small.tile([BLK, 1], f32, name=f"bias_b{b}", tag="bias")
                nc.vector.tensor_tensor(
                    out=bias[:], in0=negmx[:], in1=invt_t[:, b : b + 1], op=ALU.mult
                )
                e_t = small.tile([BLK, M], f32, name=f"e_b{b}", tag="e_t")
                sum_e = small.tile([BLK, 1], f32, name=f"sume_b{b}", tag="sum_e")
                nc.scalar.activation(
                    e_t[:],
                    att[:],
                    ACTF.Exp,
                    bias=bias[:],
                    scale=invt_t[:, b : b + 1],
                    accum_out=sum_e[:],
                )
                rec = small.tile([BLK, 1], f32, name=f"rec_b{b}", tag="rec")
                nc.vector.reciprocal(rec[:], sum_e[:])
                p_t = small.tile([BLK, M], f32, name=f"p_b{b}", tag="p_t")
                nc.vector.tensor_scalar(
                    out=p_t[:],
                    in0=e_t[:],
                    scalar1=rec[:],
                    scalar2=None,
                    op0=ALU.mult,
                )

                # ---- top-3 (descending; ties -> lowest index) ----
                mx8 = small.tile([BLK, 8], f32, name=f"mx8_b{b}", tag="mx8", bufs=3)
                nc.vector.max(mx8[:], p_t[:])
                idx8 = small.tile([BLK, 8], u32, name=f"idx8_b{b}", tag="idx8")
                nc.vector.max_index(idx8[:], mx8[:], p_t[:])

                # ---- gather indices: flat row = 32*g_local + idx ----
                # (emitted before the vals/idx output DMAs: the ACT HWDGE
                # ring is FIFO and this chain gates the dma_gather)
                flat16 = small.tile([BLK, K], i16, name=f"fl_b{b}", tag="flat16")
                nc.vector.tensor_scalar(
                    out=flat16[:],
                    in0=idx8[:, 0:K],
                    scalar1=gb_t[:, b : b + 1],
                    scalar2=None,
                    op0=ALU.add,
                )
                # wrap layout for dma_gather: entry j lives at partition
                # j % 16, slot j // 16 (replicated to all 8 stripes); we
                # gather row j = k*128 + p so the gathered row lands on
                # partition p with slot k. The 128->16 partition fold is
                # done via a DRAM bounce: one store in wrap order
                # (scr[k*128+p] = flat16[p, k]), then 8 parallel stripe
                # loads (all depending only on the store, no serial chain).
                nc.scalar.dma_start(
                    scr_d[b].rearrange("(k p) -> p k", p=BLK), flat16[:]
                )
                wrap128 = small.tile([BLK, 3 * 8], i16, name=f"wr128_b{b}", tag="wrap128")
                src16 = scr_d[b].rearrange("(s p) -> p s", p=16)
                for c in range(8):
                    nc.scalar.dma_start(wrap128[c * 16 : (c + 1) * 16, :], src16)

                nc.scalar.dma_start(vals_d[g0 : g0 + BLK, :], mx8[:, 0:K])
                nc.scalar.dma_start(
                    idx_d[g0 : g0 + BLK, :], idx8[:, 0:K].bitcast(i32)
                )

                gath = gath_p.tile([BLK, K, D], f32, name=f"gath_b{b}", tag="gath", bufs=3)
                nc.gpsimd.dma_gather(
                    gath[:],
                    wt_flat,
                    wrap128[:],
                    num_idxs=K * BLK,
                    num_idxs_reg=K * BLK,
                    elem_size=D,
                )
                pending[b] = (gath, mx8)
                if b >= 2:
                    finish_block(b - 2)

            finish_block(NBLK - 2)
            finish_block(NBLK - 1)

    nc.compile()
    _CACHE["nc"] = nc
    return nc


def host_prep(q_groups, W, temp_logit, shift_magnitude):
    """Host-side prep: per-core shards + transposed W + invtemp/gbase."""
    q = np.ascontiguousarray(np.asarray(q_groups, dtype=np.float32))
    W = np.asarray(W, dtype=np.float32)
    tl = np.asarray(temp_logit, dtype=np.float32)
    sm = np.float32(np.asarray(shift_magnitude, dtype=np.float32))

    # match the reference's f32 op chain as closely as possible
    base = (1.0 / (1.0 + np.exp(-tl.astype(np.float32)))).astype(np.float32)
    base = (base * np.float32(0.6) + np.float32(0.2)).astype(np.float32)
    temp = (base / (np.float32(1.0) + sm)).astype(np.float32)
    invt = (np.float32(1.0) / (temp + np.float32(1e-8))).astype(np.float32)

    w_t = np.ascontiguousarray(W.transpose(0, 2, 1))  # [G_TOT, M, D]
    gbase = (np.arange(G, dtype=np.float32) * np.float32(M)).astype(np.float32)

    in_maps = []
    for c in range(N_CORES):
        s = slice(c * G, (c + 1) * G)
        in_maps.append(
            {
                "q": q[s],
                "wt": w_t[s],
                "invtemp": invt[s],
                "gbase": gbase,
            }
        )
    return in_maps


def kernel(q_groups, W, temp_logit, shift_magnitude):
    global LAST_RESULTS
    from concourse.bass_utils import run_bass_kernel_spmd

    nc = build_program()
    in_maps = host_prep(q_groups, W, temp_logit, shift_magnitude)

    res = run_bass_kernel_spmd(
        nc,
        in_maps,
        core_ids=list(range(N_CORES)),
        trace=bool(os.environ.get("BASS_TRACE")),
    )
    LAST_RESULTS = res

    retrieved = np.concatenate([res.results[c]["retrieved"] for c in range(N_CORES)])
    idx = np.concatenate([res.results[c]["idx"] for c in range(N_CORES)])
    vals = np.concatenate([res.results[c]["vals"] for c in range(N_CORES)])
    return (
        retrieved.astype(np.float32, copy=False),
        idx.astype(np.int32, copy=False),
        vals.astype(np.float32, copy=False),
    )


# revision 12
# speedup vs baseline: 1.0002x; 1.0002x over previous
"""Trainium2 Bass kernel for nn_ChannelGroupedMemory (scatter_memory).

Computation per group g (G=4096 groups, D=512, M=32 slots, K=3):
    att[g, m]   = sum_d q[g, d] * W[g, d, m]
    p[g]        = softmax(att[g] / temp[g])          (temp from temp_logit, shift)
    vals, idx   = top_k(p[g], 3)                     (descending)
    retrieved[g]= sum_k vals[k] * W[g, :, idx[k]]

Strategy (8 NeuronCores, groups sharded 512/core, memory-bound):
  - Host: transpose W -> W_T [G, M, D] so each memory slot is a contiguous
    2 KiB row; precompute invtemp[g] = 1/(temp[g]+1e-8) (f32, monotone -> does
    not perturb top-k order), and gbase[g] = 32*g_local row offsets.
  - Device, per block of 128 groups (group on partition):
      * DMA W_T block [128, 16K] (64 KiB contiguous per partition), 4 quarters
      * att via 32x DVE tensor_tensor_reduce (fused mult+reduce, f32)
      * softmax: DVE max (negated), ACT Exp(scale=invtemp, bias=-max*invtemp,
        accum_out=rowsum), DVE reciprocal + tensor_scalar
      * top-3: DVE Max8 + MaxIndex (descending, lowest-index tie-break)
      * gather the 3 chosen rows per group straight from HBM via
        gpsimd.dma_gather (row index = gbase + idx), then 3 per-partition
        scalar FMAs -> retrieved
All f32 on-device: top-3 ordering is only sensitive to att error (~3e-6)
vs. min rank3/4 gap 1.8e-5 across all groups of this input distribution.
"""

import os
import sys

import numpy as np

for _p in ("/opt/trn_rl_repo",):
    if _p not in sys.path:
        sys.path.insert(0, _p)

N_CORES = 8
G_TOT, D, M, K = 4096, 512, 32, 3
G = G_TOT // N_CORES          # 512 groups per core
BLK = 128                     # groups per block (partition dim)
NBLK = G // BLK               # 4 blocks per core
MQ = 4                        # W quarters per block (8 m-slots each)

_CACHE = {}
LAST_RESULTS = None           # BassKernelResults of the most recent HW run


def build_program():
    """Build (once) the per-core Bass/Tile program. SPMD: all 8 cores run
    this same program on different group shards."""
    if "nc" in _CACHE:
        return _CACHE["nc"]

    from contextlib import ExitStack

    import concourse.bass as bass
    import concourse.tile as tile
    from concourse import bacc, library_config, mybir

    f32 = mybir.dt.float32
    i16 = mybir.dt.int16
    i32 = mybir.dt.int32
    u32 = mybir.dt.uint32
    ALU = mybir.AluOpType
    ACTF = mybir.ActivationFunctionType

    nc = bacc.Bacc(
        "TRN2",
        target_bir_lowering=False,
        debug=False,
        enable_asserts=False,
        num_devices=N_CORES,
    )

    q_d = nc.dram_tensor("q", [G, D], f32, kind="ExternalInput").ap()
    wt_d = nc.dram_tensor("wt", [G, M, D], f32, kind="ExternalInput").ap()
    invt_d = nc.dram_tensor("invtemp", [G], f32, kind="ExternalInput").ap()
    gb_d = nc.dram_tensor("gbase", [G], f32, kind="ExternalInput").ap()
    ret_d = nc.dram_tensor("retrieved", [G, D], f32, kind="ExternalOutput").ap()
    idx_d = nc.dram_tensor("idx", [G, K], i32, kind="ExternalOutput").ap()
    vals_d = nc.dram_tensor("vals", [G, K], f32, kind="ExternalOutput").ap()

    wt_flat = wt_d.rearrange("g m d -> (g m) d")
    # per-block DRAM bounce scratch for the gather-index wrap layout
    scr_d = [
        nc.dram_tensor(f"scr{b}", [K * BLK], i16).ap() for b in range(NBLK)
    ]

    with tile.TileContext(nc) as tc:
        with ExitStack() as ctx:
            nc.gpsimd.load_library(library_config.mlp)

            wq_pool = ctx.enter_context(tc.tile_pool(name="wq", bufs=2))
            small = ctx.enter_context(tc.tile_pool(name="small", bufs=2))
            const_p = ctx.enter_context(tc.tile_pool(name="const", bufs=1))
            prod_p = ctx.enter_context(tc.tile_pool(name="prod", bufs=2))
            gath_p = ctx.enter_context(tc.tile_pool(name="gath", bufs=2))
            acc_p = ctx.enter_context(tc.tile_pool(name="acc", bufs=2))

            # per-partition constants: [p, b] layouts of invtemp / gbase
            invt_t = const_p.tile([BLK, NBLK], f32, name="invt_t")
            nc.gpsimd.dma_start(
                invt_t[:], invt_d.rearrange("(b p) -> p b", p=BLK)
            )
            gb_t = const_p.tile([BLK, NBLK], f32, name="gb_t")
            nc.gpsimd.dma_start(gb_t[:], gb_d.rearrange("(b p) -> p b", p=BLK))

            # software pipeline: block b's gather-dependent FMA + retrieved
            # store are emitted during block b+1's section, so the in-order
            # DVE stream never stalls on the gather DMA chain.
            pending = {}

            def finish_block(bb):
                g0p = bb * BLK
                gath_p_, mx8_p = pending.pop(bb)
                acc = acc_p.tile([BLK, D], f32, name=f"acc_b{bb}", tag="acc")
                nc.vector.tensor_scalar(
                    out=acc[:],
                    in0=gath_p_[:, 0, :],
                    scalar1=mx8_p[:, 0:1],
                    scalar2=None,
                    op0=ALU.mult,
                )
                for k in (1, 2):
                    nc.vector.scalar_tensor_tensor(
                        out=acc[:],
                        in0=gath_p_[:, k, :],
                        scalar=mx8_p[:, k : k + 1],
                        in1=acc[:],
                        op0=ALU.mult,
                        op1=ALU.add,
                    )
                nc.gpsimd.dma_start(ret_d[g0p : g0p + BLK, :], acc[:])

            for b in range(NBLK):
                g0 = b * BLK
                # q first: every att op needs it, and the Sync HWDGE ring is
                # FIFO — q behind the W stream would stall each block's start
                q_t = small.tile([BLK, D], f32, name=f"q_b{b}", tag="q_t")
                nc.sync.dma_start(q_t[:], q_d[g0 : g0 + BLK, :])
                # ---- W block load (4 quarters of 8 m-slots: 2.1 MB DMAs,
                # alone on the HWDGE completion lanes) ----
                wq = []
                for qi in range(MQ):
                    w_t = wq_pool.tile(
                        [BLK, 8 * D], f32, name=f"w{qi}_b{b}", tag=f"wq{qi}"
                    )
                    nc.sync.dma_start(
                        w_t[:], wt_d[g0 : g0 + BLK, qi * 8 : (qi + 1) * 8, :]
                    )
                    wq.append(w_t)

                # ---- att[g, m] = <q[g], W_T[g, m, :]> ----
                att = small.tile([BLK, M], f32, name=f"att_b{b}", tag="att")
                for m in range(M):
                    # fused multiply+row-reduce on DVE:
                    # prod = (W_m + 0) * q ; att[:, m] = sum(prod)
                    # (tensor_tensor_reduce is not supported by the
                    # neuronxcc path; InstTensorScalarPtr is)
                    prod = prod_p.tile([BLK, D], f32, name=f"pr_b{b}_{m}", tag="prod")
                    nc.vector.scalar_tensor_tensor(
                        out=prod[:],
                        in0=wq[m // 8][:, (m % 8) * D : (m % 8 + 1) * D],
                        scalar=0.0,
                        in1=q_t[:],
                        op0=ALU.add,
                        op1=ALU.mult,
                        accum_out=att[:, m : m + 1],
                    )

                # ---- softmax(att * invtemp) ----
                negmx = small.tile([BLK, 1], f32, name=f"negmx_b{b}", tag="negmx")
                nc.vector.tensor_reduce(
                    out=negmx[:],
                    in_=att[:],
                    axis=mybir.AxisListType.X,
                    op=ALU.max,
                    negate=True,
                )
                bias = small.tile([BLK, 1], f32, name=f"bias_b{b}", tag="bias")
                nc.vector.tensor_tensor(
                    out=bias[:], in0=negmx[:], in1=invt_t[:, b : b + 1], op=ALU.mult
                )
                e_t = small.tile([BLK, M], f32, name=f"e_b{b}", tag="e_t")
                sum_e = small.tile([BLK, 1], f32, name=f"sume_b{b}", tag="sum_e")
                nc.scalar.activation(
                    e_t[:],
                    att[:],
                    ACTF.Exp,
                    bias=bias[:],
                    scale=invt_t[:, b : b + 1],
                    accum_out=sum_e[:],
                )
                rec = small.tile([BLK, 1], f32, name=f"rec_b{b}", tag="rec")
                nc.vector.reciprocal(rec[:], sum_e[:])
                p_t = small.tile([BLK, M], f32, name=f"p_b{b}", tag="p_t")
                nc.vector.tensor_scalar(
                    out=p_t[:],
                    in0=e_t[:],
                    scalar1=rec[:],
                    scalar2=None,
                    op0=ALU.mult,
                )

                # ---- top-3 (descending; ties -> lowest index) ----
                mx8 = small.tile([BLK, 8], f32, name=f"mx8_b{b}", tag="mx8", bufs=3)
                nc.vector.max(mx8[:], p_t[:])
                idx8 = small.tile([BLK, 8], u32, name=f"idx8_b{b}", tag="idx8")
                nc.vector.max_index(idx8[:], mx8[:], p_t[:])

                # ---- gather indices: flat row = 32*g_local + idx ----
                # (emitted before the vals/idx output DMAs: the ACT HWDGE
                # ring is FIFO and this chain gates the dma_gather)
                flat16 = small.tile([BLK, K], i16, name=f"fl_b{b}", tag="flat16")
                nc.vector.tensor_scalar(
                    out=flat16[:],
                    in0=idx8[:, 0:K],
                    scalar1=gb_t[:, b : b + 1],
                    scalar2=None,
                    op0=ALU.add,
                )
                # wrap layout for dma_gather: entry j lives at partition
                # j % 16, slot j // 16 (replicated to all 8 stripes); we
                # gather row j = k*128 + p so the gathered row lands on
                # partition p with slot k. The 128->16 partition fold is
                # done via a DRAM bounce: one store in wrap order
                # (scr[k*128+p] = flat16[p, k]), then 8 parallel stripe
                # loads (all depending only on the store, no serial chain).
                nc.gpsimd.dma_start(
                    scr_d[b].rearrange("(k p) -> p k", p=BLK), flat16[:]
                )
                wrap128 = small.tile([BLK, 3 * 8], i16, name=f"wr128_b{b}", tag="wrap128")
                src16 = scr_d[b].rearrange("(s p) -> p s", p=16)
                for c in range(8):
                    nc.gpsimd.dma_start(wrap128[c * 16 : (c + 1) * 16, :], src16)

                nc.gpsimd.dma_start(vals_d[g0 : g0 + BLK, :], mx8[:, 0:K])
                nc.gpsimd.dma_start(
                    idx_d[g0 : g0 + BLK, :], idx8[:, 0:K].bitcast(i32)
                )

                gath = gath_p.tile([BLK, K, D], f32, name=f"gath_b{b}", tag="gath", bufs=3)
                nc.gpsimd.dma_gather(
                    gath[:],
                    wt_flat,
                    wrap128[:],
                    num_idxs=K * BLK,
                    num_idxs_reg=K * BLK,
                    elem_size=D,
                )
                pending[b] = (gath, mx8)
                if b >= 2:
                    finish_block(b - 2)

            finish_block(NBLK - 2)
            finish_block(NBLK - 1)

    nc.compile()
    _CACHE["nc"] = nc
    return nc


def host_prep(q_groups, W, temp_logit, shift_magnitude):
    """Host-side prep: per-core shards + transposed W + invtemp/gbase."""
    q = np.ascontiguousarray(np.asarray(q_groups, dtype=np.float32))
    W = np.asarray(W, dtype=np.float32)
    tl = np.asarray(temp_logit, dtype=np.float32)
    sm = np.float32(np.asarray(shift_magnitude, dtype=np.float32))

    # match the reference's f32 op chain as closely as possible
    base = (1.0 / (1.0 + np.exp(-tl.astype(np.float32)))).astype(np.float32)
    base = (base * np.float32(0.6) + np.float32(0.2)).astype(np.float32)
    temp = (base / (np.float32(1.0) + sm)).astype(np.float32)
    invt = (np.float32(1.0) / (temp + np.float32(1e-8))).astype(np.float32)

    w_t = np.ascontiguousarray(W.transpose(0, 2, 1))  # [G_TOT, M, D]
    gbase = (np.arange(G, dtype=np.float32) * np.float32(M)).astype(np.float32)

    in_maps = []
    for c in range(N_CORES):
        s = slice(c * G, (c + 1) * G)
        in_maps.append(
            {
                "q": q[s],
                "wt": w_t[s],
                "invtemp": invt[s],
                "gbase": gbase,
            }
        )
    return in_maps


def kernel(q_groups, W, temp_logit, shift_magnitude):
    global LAST_RESULTS
    from concourse.bass_utils import run_bass_kernel_spmd

    nc = build_program()
    in_maps = host_prep(q_groups, W, temp_logit, shift_magnitude)

    res = run_bass_kernel_spmd(
        nc,
        in_maps,
        core_ids=list(range(N_CORES)),
        trace=bool(os.environ.get("BASS_TRACE")),
    )
    LAST_RESULTS = res

    retrieved = np.concatenate([res.results[c]["retrieved"] for c in range(N_CORES)])
    idx = np.concatenate([res.results[c]["idx"] for c in range(N_CORES)])
    vals = np.concatenate([res.results[c]["vals"] for c in range(N_CORES)])
    return (
        retrieved.astype(np.float32, copy=False),
        idx.astype(np.int32, copy=False),
        vals.astype(np.float32, copy=False),
    )


# revision 13
# speedup vs baseline: 1.3418x; 1.3416x over previous
"""Trainium2 Bass kernel for nn_ChannelGroupedMemory (scatter_memory).

Computation per group g (G=4096 groups, D=512, M=32 slots, K=3):
    att[g, m]   = sum_d q[g, d] * W[g, d, m]
    p[g]        = softmax(att[g] / temp[g])          (temp from temp_logit, shift)
    vals, idx   = top_k(p[g], 3)                     (descending)
    retrieved[g]= sum_k vals[k] * W[g, :, idx[k]]

Strategy (8 NeuronCores, groups sharded 512/core, memory-bound):
  - Host: transpose W -> W_T [G, M, D] so each memory slot is a contiguous
    2 KiB row; precompute invtemp[g] = 1/(temp[g]+1e-8) (f32, monotone -> does
    not perturb top-k order), and gbase[g] = 32*g_local row offsets.
  - Device, per block of 128 groups (group on partition):
      * DMA W_T block [128, 16K] (64 KiB contiguous per partition), 4 quarters
      * att via 32x DVE tensor_tensor_reduce (fused mult+reduce, f32)
      * softmax: DVE max (negated), ACT Exp(scale=invtemp, bias=-max*invtemp,
        accum_out=rowsum), DVE reciprocal + tensor_scalar
      * top-3: DVE Max8 + MaxIndex (descending, lowest-index tie-break)
      * gather the 3 chosen rows per group straight from HBM via
        gpsimd.dma_gather (row index = gbase + idx), then 3 per-partition
        scalar FMAs -> retrieved
All f32 on-device: top-3 ordering is only sensitive to att error (~3e-6)
vs. min rank3/4 gap 1.8e-5 across all groups of this input distribution.
"""

import os
import sys

import numpy as np

for _p in ("/opt/trn_rl_repo",):
    if _p not in sys.path:
        sys.path.insert(0, _p)

N_CORES = 8
G_TOT, D, M, K = 4096, 512, 32, 3
G = G_TOT // N_CORES          # 512 groups per core
BLK = 128                     # groups per block (partition dim)
NBLK = G // BLK               # 4 blocks per core
MQ = 4                        # W quarters per block (8 m-slots each)

_CACHE = {}
LAST_RESULTS = None           # BassKernelResults of the most recent HW run


def build_program():
    """Build (once) the per-core Bass/Tile program. SPMD: all 8 cores run
    this same program on different group shards."""
    if "nc" in _CACHE:
        return _CACHE["nc"]

    from contextlib import ExitStack

    import concourse.bass as bass
    import concourse.tile as tile
    from concourse import bacc, library_config, mybir

    f32 = mybir.dt.float32
    i16 = mybir.dt.int16
    i32 = mybir.dt.int32
    u32 = mybir.dt.uint32
    ALU = mybir.AluOpType
    ACTF = mybir.ActivationFunctionType

    nc = bacc.Bacc(
        "TRN2",
        target_bir_lowering=False,
        debug=False,
        enable_asserts=False,
        num_devices=N_CORES,
    )

    q_d = nc.dram_tensor("q", [G, D], f32, kind="ExternalInput").ap()
    wt_d = nc.dram_tensor("wt", [G, M, D], f32, kind="ExternalInput").ap()
    invt_d = nc.dram_tensor("invtemp", [G], f32, kind="ExternalInput").ap()
    gb_d = nc.dram_tensor("gbase", [G], f32, kind="ExternalInput").ap()
    ret_d = nc.dram_tensor("retrieved", [G, D], f32, kind="ExternalOutput").ap()
    idx_d = nc.dram_tensor("idx", [G, K], i32, kind="ExternalOutput").ap()
    vals_d = nc.dram_tensor("vals", [G, K], f32, kind="ExternalOutput").ap()

    wt_flat = wt_d.rearrange("g m d -> (g m) d")
    # per-block DRAM bounce scratch for the gather-index wrap layout
    scr_d = [
        nc.dram_tensor(f"scr{b}", [K * BLK], i16).ap() for b in range(NBLK)
    ]

    with tile.TileContext(nc) as tc:
        with ExitStack() as ctx:
            nc.gpsimd.load_library(library_config.mlp)

            wq_pool = ctx.enter_context(tc.tile_pool(name="wq", bufs=2))
            small = ctx.enter_context(tc.tile_pool(name="small", bufs=2))
            const_p = ctx.enter_context(tc.tile_pool(name="const", bufs=1))
            prod_p = ctx.enter_context(tc.tile_pool(name="prod", bufs=2))
            gath_p = ctx.enter_context(tc.tile_pool(name="gath", bufs=2))
            acc_p = ctx.enter_context(tc.tile_pool(name="acc", bufs=2))

            # per-partition constants: [p, b] layouts of invtemp / gbase
            invt_t = const_p.tile([BLK, NBLK], f32, name="invt_t")
            nc.gpsimd.dma_start(
                invt_t[:], invt_d.rearrange("(b p) -> p b", p=BLK)
            )
            gb_t = const_p.tile([BLK, NBLK], f32, name="gb_t")
            nc.gpsimd.dma_start(gb_t[:], gb_d.rearrange("(b p) -> p b", p=BLK))

            # software pipeline: block b's gather-dependent FMA + retrieved
            # store are emitted during block b+1's section, so the in-order
            # DVE stream never stalls on the gather DMA chain.
            pending = {}

            def finish_block(bb):
                g0p = bb * BLK
                gath_p_, mx8_p = pending.pop(bb)
                acc = acc_p.tile([BLK, D], f32, name=f"acc_b{bb}", tag="acc")
                nc.vector.tensor_scalar(
                    out=acc[:],
                    in0=gath_p_[:, 0, :],
                    scalar1=mx8_p[:, 0:1],
                    scalar2=None,
                    op0=ALU.mult,
                )
                for k in (1, 2):
                    nc.vector.scalar_tensor_tensor(
                        out=acc[:],
                        in0=gath_p_[:, k, :],
                        scalar=mx8_p[:, k : k + 1],
                        in1=acc[:],
                        op0=ALU.mult,
                        op1=ALU.add,
                    )
                nc.gpsimd.dma_start(ret_d[g0p : g0p + BLK, :], acc[:])

            for b in range(NBLK):
                g0 = b * BLK
                # q first: every att op needs it, and the Sync HWDGE ring is
                # FIFO — q behind the W stream would stall each block's start
                q_t = small.tile([BLK, D], f32, name=f"q_b{b}", tag="q_t")
                nc.sync.dma_start(q_t[:], q_d[g0 : g0 + BLK, :])
                # ---- W block load (4 quarters of 8 m-slots: 2.1 MB DMAs,
                # alone on the HWDGE completion lanes) ----
                wq = []
                for qi in range(MQ):
                    w_t = wq_pool.tile(
                        [BLK, 8 * D], f32, name=f"w{qi}_b{b}", tag=f"wq{qi}"
                    )
                    nc.sync.dma_start(
                        w_t[:], wt_d[g0 : g0 + BLK, qi * 8 : (qi + 1) * 8, :]
                    )
                    wq.append(w_t)

                # ---- att[g, m] = <q[g], W_T[g, m, :]> ----
                att = small.tile([BLK, M], f32, name=f"att_b{b}", tag="att")
                for m in range(M):
                    # fused multiply+row-reduce on DVE:
                    # prod = (W_m + 0) * q ; att[:, m] = sum(prod)
                    # (tensor_tensor_reduce is not supported by the
                    # neuronxcc path; InstTensorScalarPtr is)
                    prod = prod_p.tile([BLK, D], f32, name=f"pr_b{b}_{m}", tag="prod")
                    nc.vector.scalar_tensor_tensor(
                        out=prod[:],
                        in0=wq[m // 8][:, (m % 8) * D : (m % 8 + 1) * D],
                        scalar=0.0,
                        in1=q_t[:],
                        op0=ALU.add,
                        op1=ALU.mult,
                        accum_out=att[:, m : m + 1],
                    )

                # ---- softmax(att * invtemp) ----
                negmx = small.tile([BLK, 1], f32, name=f"negmx_b{b}", tag="negmx")
                nc.vector.tensor_reduce(
                    out=negmx[:],
                    in_=att[:],
                    axis=mybir.AxisListType.X,
                    op=ALU.max,
                    negate=True,
                )
                bias = small.tile([BLK, 1], f32, name=f"bias_b{b}", tag="bias")
                nc.vector.tensor_tensor(
                    out=bias[:], in0=negmx[:], in1=invt_t[:, b : b + 1], op=ALU.mult
                )
                e_t = small.tile([BLK, M], f32, name=f"e_b{b}", tag="e_t")
                sum_e = small.tile([BLK, 1], f32, name=f"sume_b{b}", tag="sum_e")
                nc.scalar.activation(
                    e_t[:],
                    att[:],
                    ACTF.Exp,
                    bias=bias[:],
                    scale=invt_t[:, b : b + 1],
                    accum_out=sum_e[:],
                )
                rec = small.tile([BLK, 1], f32, name=f"rec_b{b}", tag="rec")
                nc.vector.reciprocal(rec[:], sum_e[:])
                p_t = small.tile([BLK, M], f32, name=f"p_b{b}", tag="p_t")
                nc.vector.tensor_scalar(
                    out=p_t[:],
                    in0=e_t[:],
                    scalar1=rec[:],
                    scalar2=None,
                    op0=ALU.mult,
                )

                # ---- top-3 (descending; ties -> lowest index) ----
                mx8 = small.tile([BLK, 8], f32, name=f"mx8_b{b}", tag="mx8", bufs=3)
                nc.vector.max(mx8[:], p_t[:])
                idx8 = small.tile([BLK, 8], u32, name=f"idx8_b{b}", tag="idx8")
                nc.vector.max_index(idx8[:], mx8[:], p_t[:])

                # ---- gather indices: flat row = 32*g_local + idx ----
                # (emitted before the vals/idx output DMAs: the ACT HWDGE
                # ring is FIFO and this chain gates the dma_gather)
                flat16 = small.tile([BLK, K], i16, name=f"fl_b{b}", tag="flat16")
                nc.vector.tensor_scalar(
                    out=flat16[:],
                    in0=idx8[:, 0:K],
                    scalar1=gb_t[:, b : b + 1],
                    scalar2=None,
                    op0=ALU.add,
                )
                # wrap layout for dma_gather: entry j lives at partition
                # j % 16, slot j // 16 (replicated to all 8 stripes); we
                # gather row j = k*128 + p so the gathered row lands on
                # partition p with slot k. All-SBUF two-hop fold (DRAM
                # bounces stall 20-40us behind the W stream's HBM traffic):
                # 8 parallel shuffle DMAs fold 128->16 partitions, then 8
                # parallel replicate DMAs fan out to every 16-row stripe.
                wrap16 = small.tile([16, 3 * 8], i16, name=f"wr16_b{b}", tag="wrap16")
                w16v = wrap16[0:16, :].rearrange("p (k e) -> p k e", e=8)
                for ph in range(8):
                    nc.gpsimd.dma_start(
                        w16v[:, :, ph : ph + 1],
                        flat16[ph * 16 : (ph + 1) * 16, :],
                    )
                wrap128 = small.tile([BLK, 3 * 8], i16, name=f"wr128_b{b}", tag="wrap128")
                for c in range(8):
                    nc.gpsimd.dma_start(
                        wrap128[c * 16 : (c + 1) * 16, :], wrap16[0:16, :]
                    )

                nc.gpsimd.dma_start(vals_d[g0 : g0 + BLK, :], mx8[:, 0:K])
                nc.gpsimd.dma_start(
                    idx_d[g0 : g0 + BLK, :], idx8[:, 0:K].bitcast(i32)
                )

                gath = gath_p.tile([BLK, K, D], f32, name=f"gath_b{b}", tag="gath", bufs=3)
                nc.gpsimd.dma_gather(
                    gath[:],
                    wt_flat,
                    wrap128[:],
                    num_idxs=K * BLK,
                    num_idxs_reg=K * BLK,
                    elem_size=D,
                )
                pending[b] = (gath, mx8)
                if b >= 2:
                    finish_block(b - 2)

            finish_block(NBLK - 2)
            finish_block(NBLK - 1)

    nc.compile()
    _CACHE["nc"] = nc
    return nc


def host_prep(q_groups, W, temp_logit, shift_magnitude):
    """Host-side prep: per-core shards + transposed W + invtemp/gbase."""
    q = np.ascontiguousarray(np.asarray(q_groups, dtype=np.float32))
    W = np.asarray(W, dtype=np.float32)
    tl = np.asarray(temp_logit, dtype=np.float32)
    sm = np.float32(np.asarray(shift_magnitude, dtype=np.float32))

    # match the reference's f32 op chain as closely as possible
    base = (1.0 / (1.0 + np.exp(-tl.astype(np.float32)))).astype(np.float32)
    base = (base * np.float32(0.6) + np.float32(0.2)).astype(np.float32)
    temp = (base / (np.float32(1.0) + sm)).astype(np.float32)
    invt = (np.float32(1.0) / (temp + np.float32(1e-8))).astype(np.float32)

    w_t = np.ascontiguousarray(W.transpose(0, 2, 1))  # [G_TOT, M, D]
    gbase = (np.arange(G, dtype=np.float32) * np.float32(M)).astype(np.float32)

    in_maps = []
    for c in range(N_CORES):
        s = slice(c * G, (c + 1) * G)
        in_maps.append(
            {
                "q": q[s],
                "wt": w_t[s],
                "invtemp": invt[s],
                "gbase": gbase,
            }
        )
    return in_maps


def kernel(q_groups, W, temp_logit, shift_magnitude):
    global LAST_RESULTS
    from concourse.bass_utils import run_bass_kernel_spmd

    nc = build_program()
    in_maps = host_prep(q_groups, W, temp_logit, shift_magnitude)

    res = run_bass_kernel_spmd(
        nc,
        in_maps,
        core_ids=list(range(N_CORES)),
        trace=bool(os.environ.get("BASS_TRACE")),
    )
    LAST_RESULTS = res

    retrieved = np.concatenate([res.results[c]["retrieved"] for c in range(N_CORES)])
    idx = np.concatenate([res.results[c]["idx"] for c in range(N_CORES)])
    vals = np.concatenate([res.results[c]["vals"] for c in range(N_CORES)])
    return (
        retrieved.astype(np.float32, copy=False),
        idx.astype(np.int32, copy=False),
        vals.astype(np.float32, copy=False),
    )
